# revision 3
# baseline (speedup 1.0000x reference)
"""CP-decomposed conv (pointwise -> depthwise-h -> depthwise-w -> pointwise)
as a Bass/Tile kernel on 8 TRN2 NeuronCores.

Strategy (v2):
  - Data-parallel over batch: 32 images -> 4 per core, no collectives.
  - fp16 wire format for x and out (halves HBM bytes; HBM floor ~154us/core).
  - Per image, 2 row-strips of S=47 output rows (49 input rows with halo).
  - HYBRID h-conv schedule, tuned so PE / DVE / ACT all sit below the DMA
    roofline:
      * FOLDED strips: h-conv folded into the C->R pointwise matmul
        (6 accumulating fp16 matmuls per PSUM tile; 3x the PE work of a
        plain pointwise but zero DVE work), then the w-conv runs straight
        out of PSUM (ACT mul + 2 DVE STT per tile).
      * UNFOLDED strips: plain pointwise C->R (PE 3x cheaper), PSUM->SBUF
        cast copy on ACT, then h-conv and w-conv as fp16 SBUF ops on DVE
        where tensor_scalar runs at 4x and STT at 2x.
  - Final projection R->F: one 128x128 fp16 matmul per 512-col half of a
    2-bank PSUM tile; PSUM->SBUF cast copies split ACT/DVE by a weighted
    round-robin to balance the two engines.
  - Input DMAs on GpSimd SWDGE (idle engine), output DMAs on SP HWDGE;
    one ~2.4MB input DMA and one ~4.5MB output DMA per strip.
"""

import sys
import numpy as np

for _p in ("/opt/trn_rl_repo",):
    if _p not in sys.path:
        sys.path.insert(0, _p)

B, C, H, W = 32, 256, 96, 96
F, FH, FW, R = 512, 3, 3, 128
OH, OW = H - FH + 1, W - FW + 1  # 94, 94
NCORES = 8
BLOC = B // NCORES  # 4 images per core

S = 47                       # output rows per strip
STRIPS = [(0, S), (S, S)]    # per image
NRI = S + 2                  # input rows per strip (halo)

# fold schedule over the 8 (image, strip) pairs per core:
# 1 = h-conv folded into stage-A matmuls (PE-heavy),
# 0 = h-conv on DVE in fp16 (vector-heavy).
FOLD = [1, 1, 0, 1, 1, 0, 1, 1]

# fraction of stage-D PSUM->SBUF copy elements sent to DVE (rest on ACT)
DVE_COPY_FRAC = 0.27

# row tiles within a folded strip (rows per PSUM tile, <= 10 to fit 2 banks)
FOLD_ROWTILES = [10, 10, 10, 10, 7]
# flat col tiles for a 2-bank (1024 fp32) PSUM tile
def _tiles(total, size):
    out, c0 = [], 0
    while c0 < total:
        t = min(size, total - c0)
        out.append((c0, t))
        c0 += t
    return out


def _halves(n):
    if n <= 512:
        return [(0, n)]
    return [(0, 512), (512, n - 512)]


_NC_CACHE = {}


def _build_nc():
    import concourse.bacc as bacc
    import concourse.mybir as mybir
    import concourse.tile as tile

    f32 = mybir.dt.float32
    f16 = mybir.dt.float16
    mult = mybir.AluOpType.mult
    add = mybir.AluOpType.add

    nc = bacc.Bacc("TRN2", target_bir_lowering=False, debug=True)

    xd = nc.dram_tensor("x", [BLOC, C, H, W], f16, kind="ExternalInput")
    # wt packs 12 [128,128] weight tiles:
    #   [0:6]  folded stage-A:  [h*2+ch, c', r] = f3[ch*128+c', r] * f1[h, r]
    #   [6:8]  plain stage-A:   [6+ch,   c', r] = f3[ch*128+c', r]
    #   [8:12] stage-D:         [8+fc,   r, f'] = f0[fc*128+f', r]
    wtd = nc.dram_tensor("wt", [12, 128, 128], f16, kind="ExternalInput")
    # wsc[r, 0:3] = f1[h, r]; wsc[r, 3:6] = f2[w, r]
    wscd = nc.dram_tensor("wsc", [R, 8], f32, kind="ExternalInput")
    od = nc.dram_tensor("out", [BLOC, F, OH, OW], f16, kind="ExternalOutput")

    with tile.TileContext(nc) as tc:
        with (
            tc.tile_pool(name="wpool", bufs=1) as wpool,
            tc.tile_pool(name="xs", bufs=3) as xs_pool,
            tc.tile_pool(name="y1p", bufs=2) as y1_pool,
            tc.tile_pool(name="y2p", bufs=2) as y2_pool,
            tc.tile_pool(name="y3p", bufs=2) as y3_pool,
            tc.tile_pool(name="osb", bufs=2) as osb_pool,
            tc.tile_pool(name="ps", bufs=4, space="PSUM") as ps_pool,
        ):
            wsc_sb = wpool.tile([128, 8], f32)
            nc.sync.dma_start(wsc_sb[:], wscd[:])
            wt_sb = wpool.tile([128, 12, 128], f16)
            nc.sync.dma_start(wt_sb[:], wtd.ap().rearrange("t p c -> p t c"))

            # weighted round-robin for stage-D copy engine assignment
            dve_credit = [0.0]

            def d_copy(dst, src):
                dve_credit[0] += DVE_COPY_FRAC
                if dve_credit[0] >= 1.0:
                    dve_credit[0] -= 1.0
                    nc.vector.tensor_copy(dst, src)
                else:
                    nc.scalar.copy(dst, src)

            ordinal = 0
            for b in range(BLOC):
                for i0, _S in STRIPS:
                    folded = FOLD[ordinal]
                    ordinal += 1

                    xs_t = xs_pool.tile([128, 2, NRI * W], f16)
                    nc.gpsimd.dma_start(
                        xs_t[:],
                        xd[b, :, i0 : i0 + NRI, :].rearrange(
                            "(c p) h w -> p c (h w)", p=128
                        ),
                    )

                    y3_t = y3_pool.tile([128, S * OW], f16)

                    if folded:
                        r0 = 0
                        for nr in FOLD_ROWTILES:
                            ncols = nr * W
                            pa = ps_pool.tile([128, 1024], f32, tag="pt")
                            for c0, cn in _halves(ncols):
                                k = 0
                                for h in range(FH):
                                    for ch in range(2):
                                        nc.tensor.matmul(
                                            pa[:, c0 : c0 + cn],
                                            wt_sb[:, h * 2 + ch, :],
                                            xs_t[
                                                :,
                                                ch,
                                                (r0 + h) * W + c0 : (r0 + h) * W
                                                + c0
                                                + cn,
                                            ],
                                            start=(k == 0),
                                            stop=(k == 5),
                                        )
                                        k += 1
                            pav = pa[:, 0:ncols].rearrange(
                                "p (r w) -> p r w", w=W
                            )
                            dst = y3_t[:, r0 * OW : (r0 + nr) * OW].rearrange(
                                "p (r j) -> p r j", j=OW
                            )
                            nc.scalar.mul(
                                dst, pav[:, :, 0:OW], wsc_sb[:, 3:4]
                            )
                            nc.vector.scalar_tensor_tensor(
                                dst, pav[:, :, 1 : 1 + OW], wsc_sb[:, 4:5],
                                dst, op0=mult, op1=add,
                            )
                            nc.vector.scalar_tensor_tensor(
                                dst, pav[:, :, 2 : 2 + OW], wsc_sb[:, 5:6],
                                dst, op0=mult, op1=add,
                            )
                            r0 += nr
                    else:
                        y1_t = y1_pool.tile([128, NRI * W], f16)
                        for t0, tn in _tiles(NRI * W, 1024):
                            pa = ps_pool.tile([128, 1024], f32, tag="pt")
                            for c0, cn in _halves(tn):
                                for ch in range(2):
                                    nc.tensor.matmul(
                                        pa[:, c0 : c0 + cn],
                                        wt_sb[:, 6 + ch, :],
                                        xs_t[:, ch, t0 + c0 : t0 + c0 + cn],
                                        start=(ch == 0),
                                        stop=(ch == 1),
                                    )
                            nc.scalar.copy(y1_t[:, t0 : t0 + tn], pa[:, 0:tn])
                        # h-conv in fp16 on DVE (4x mul, 2x STT)
                        y2_t = y2_pool.tile([128, S * W], f16)
                        nc.vector.tensor_scalar_mul(
                            y2_t[:], y1_t[:, 0 : S * W], wsc_sb[:, 0:1]
                        )
                        nc.vector.scalar_tensor_tensor(
                            y2_t[:], y1_t[:, W : W + S * W], wsc_sb[:, 1:2],
                            y2_t[:], op0=mult, op1=add,
                        )
                        nc.vector.scalar_tensor_tensor(
                            y2_t[:], y1_t[:, 2 * W : 2 * W + S * W],
                            wsc_sb[:, 2:3], y2_t[:], op0=mult, op1=add,
                        )
                        # w-conv in fp16 on DVE
                        y2v = y2_t.rearrange("p (r w) -> p r w", w=W)
                        y3v = y3_t.rearrange("p (r j) -> p r j", j=OW)
                        nc.vector.tensor_scalar_mul(
                            y3v, y2v[:, :, 0:OW], wsc_sb[:, 3:4]
                        )
                        nc.vector.scalar_tensor_tensor(
                            y3v, y2v[:, :, 1 : 1 + OW], wsc_sb[:, 4:5], y3v,
                            op0=mult, op1=add,
                        )
                        nc.vector.scalar_tensor_tensor(
                            y3v, y2v[:, :, 2 : 2 + OW], wsc_sb[:, 5:6], y3v,
                            op0=mult, op1=add,
                        )

                    # stage D: projection R->F over flat col tiles of y3
                    ot = osb_pool.tile([128, 4, S * OW], f16)
                    for fc in range(4):
                        for c0, cn in _tiles(S * OW, 1024):
                            pd = ps_pool.tile([128, 1024], f32, tag="pt")
                            for h0, hn in _halves(cn):
                                nc.tensor.matmul(
                                    pd[:, h0 : h0 + hn],
                                    wt_sb[:, 8 + fc, :],
                                    y3_t[:, c0 + h0 : c0 + h0 + hn],
                                    start=True,
                                    stop=True,
                                )
                            d_copy(ot[:, fc, c0 : c0 + cn], pd[:, 0:cn])
                    nc.sync.dma_start(
                        od[b, :, i0 : i0 + S, :].rearrange(
                            "(c p) i j -> p c (i j)", p=128
                        ),
                        ot[:],
                    )

    nc.compile()
    return nc


def _get_nc():
    if "nc" not in _NC_CACHE:
        _NC_CACHE["nc"] = _build_nc()
    return _NC_CACHE["nc"]


def _prep_weights(factor0, factor1, factor2, factor3):
    wa = (factor3[None, :, :] * factor1[:, None, :]).reshape(FH, 2, 128, R)
    w3 = factor3.reshape(2, 128, R)
    w0 = factor0.reshape(4, 128, R).transpose(0, 2, 1)
    wt = np.concatenate(
        [wa.reshape(6, 128, R), w3, w0], axis=0
    ).astype(np.float16)
    wt = np.ascontiguousarray(wt)
    wsc = np.zeros((R, 8), dtype=np.float32)
    wsc[:, 0:3] = factor1.T
    wsc[:, 3:6] = factor2.T
    return wt, wsc


def _prep_x(x):
    return np.ascontiguousarray(x).astype(np.float16)


def _make_in_maps(x, factor0, factor1, factor2, factor3):
    wt, wsc = _prep_weights(factor0, factor1, factor2, factor3)
    x16 = _prep_x(x)
    return [
        {"x": x16[c * BLOC : (c + 1) * BLOC], "wt": wt, "wsc": wsc}
        for c in range(NCORES)
    ]


def kernel(x, factor0, factor1, factor2, factor3):
    from concourse import bass_utils

    x = np.asarray(x, dtype=np.float32)
    factor0 = np.asarray(factor0, dtype=np.float32)
    factor1 = np.asarray(factor1, dtype=np.float32)
    factor2 = np.asarray(factor2, dtype=np.float32)
    factor3 = np.asarray(factor3, dtype=np.float32)

    in_maps = _make_in_maps(x, factor0, factor1, factor2, factor3)
    nc = _get_nc()
    res = bass_utils.run_bass_kernel_spmd(nc, in_maps, list(range(NCORES)))
    out = np.concatenate(
        [res.results[c]["out"] for c in range(NCORES)], axis=0
    )
    return out.astype(np.float32)


# revision 4
# speedup vs baseline: 1.1407x; 1.1407x over previous
"""CP-decomposed conv (pointwise -> depthwise-h -> depthwise-w -> pointwise)
as a Bass/Tile kernel on 8 TRN2 NeuronCores.

Strategy (v2):
  - Data-parallel over batch: 32 images -> 4 per core, no collectives.
  - fp16 wire format for x and out (halves HBM bytes; HBM floor ~154us/core).
  - Per image, 2 row-strips of S=47 output rows (49 input rows with halo).
  - HYBRID h-conv schedule, tuned so PE / DVE / ACT all sit below the DMA
    roofline:
      * FOLDED strips: h-conv folded into the C->R pointwise matmul
        (6 accumulating fp16 matmuls per PSUM tile; 3x the PE work of a
        plain pointwise but zero DVE work), then the w-conv runs straight
        out of PSUM (ACT mul + 2 DVE STT per tile).
      * UNFOLDED strips: plain pointwise C->R (PE 3x cheaper), PSUM->SBUF
        cast copy on ACT, then h-conv and w-conv as fp16 SBUF ops on DVE
        where tensor_scalar runs at 4x and STT at 2x.
  - Final projection R->F: one 128x128 fp16 matmul per 512-col half of a
    2-bank PSUM tile; PSUM->SBUF cast copies split ACT/DVE by a weighted
    round-robin to balance the two engines.
  - Input DMAs on GpSimd SWDGE (idle engine), output DMAs on SP HWDGE;
    one ~2.4MB input DMA and one ~4.5MB output DMA per strip.
"""

import sys
import numpy as np

for _p in ("/opt/trn_rl_repo",):
    if _p not in sys.path:
        sys.path.insert(0, _p)

B, C, H, W = 32, 256, 96, 96
F, FH, FW, R = 512, 3, 3, 128
OH, OW = H - FH + 1, W - FW + 1  # 94, 94
NCORES = 8
BLOC = B // NCORES  # 4 images per core

S = 47                       # output rows per strip
STRIPS = [(0, S), (S, S)]    # per image
NRI = S + 2                  # input rows per strip (halo)

# fold schedule over the 8 (image, strip) pairs per core:
# 1 = h-conv folded into stage-A matmuls (PE-heavy),
# 0 = h-conv on DVE in fp16 (vector-heavy).
FOLD = [1, 1, 1, 1, 1, 1, 1, 1]

# fraction of stage-D PSUM->SBUF copy elements sent to DVE (rest on ACT)
DVE_COPY_FRAC = 0.28

# row tiles within a folded strip (rows per PSUM tile, <= 10 to fit 2 banks)
FOLD_ROWTILES = [10, 10, 10, 10, 7]
# flat col tiles for a 2-bank (1024 fp32) PSUM tile
def _tiles(total, size):
    out, c0 = [], 0
    while c0 < total:
        t = min(size, total - c0)
        out.append((c0, t))
        c0 += t
    return out


def _halves(n):
    if n <= 512:
        return [(0, n)]
    return [(0, 512), (512, n - 512)]


_NC_CACHE = {}


def _build_nc():
    import concourse.bacc as bacc
    import concourse.mybir as mybir
    import concourse.tile as tile

    f32 = mybir.dt.float32
    f16 = mybir.dt.float16
    mult = mybir.AluOpType.mult
    add = mybir.AluOpType.add

    nc = bacc.Bacc("TRN2", target_bir_lowering=False, debug=True)

    xd = nc.dram_tensor("x", [BLOC, C, H, W], f16, kind="ExternalInput")
    # wt packs 12 [128,128] weight tiles:
    #   [0:6]  folded stage-A:  [h*2+ch, c', r] = f3[ch*128+c', r] * f1[h, r]
    #   [6:8]  plain stage-A:   [6+ch,   c', r] = f3[ch*128+c', r]
    #   [8:12] stage-D:         [8+fc,   r, f'] = f0[fc*128+f', r]
    wtd = nc.dram_tensor("wt", [12, 128, 128], f16, kind="ExternalInput")
    # wsc[r, 0:3] = f1[h, r]; wsc[r, 3:6] = f2[w, r]
    wscd = nc.dram_tensor("wsc", [R, 8], f32, kind="ExternalInput")
    od = nc.dram_tensor("out", [BLOC, F, OH, OW], f16, kind="ExternalOutput")

    with tile.TileContext(nc) as tc:
        with (
            tc.tile_pool(name="wpool", bufs=1) as wpool,
            tc.tile_pool(name="xs", bufs=3) as xs_pool,
            tc.tile_pool(name="y1p", bufs=2) as y1_pool,
            tc.tile_pool(name="y2p", bufs=2) as y2_pool,
            tc.tile_pool(name="y3p", bufs=2) as y3_pool,
            tc.tile_pool(name="osb", bufs=2) as osb_pool,
            tc.tile_pool(name="ps", bufs=4, space="PSUM") as ps_pool,
        ):
            wsc_sb = wpool.tile([128, 8], f32)
            nc.sync.dma_start(wsc_sb[:], wscd[:])
            wt_sb = wpool.tile([128, 12, 128], f16)
            nc.sync.dma_start(wt_sb[:], wtd.ap().rearrange("t p c -> p t c"))

            # weighted round-robin for stage-D copy engine assignment
            dve_credit = [0.0]

            def d_copy(dst, src):
                dve_credit[0] += DVE_COPY_FRAC
                if dve_credit[0] >= 1.0:
                    dve_credit[0] -= 1.0
                    nc.vector.tensor_copy(dst, src)
                else:
                    nc.scalar.copy(dst, src)

            ordinal = 0
            for b in range(BLOC):
                for i0, _S in STRIPS:
                    folded = FOLD[ordinal]
                    ordinal += 1

                    xs_t = xs_pool.tile([128, 2, NRI * W], f16)
                    nc.gpsimd.dma_start(
                        xs_t[:],
                        xd[b, :, i0 : i0 + NRI, :].rearrange(
                            "(c p) h w -> p c (h w)", p=128
                        ),
                    )

                    y3_t = y3_pool.tile([128, S * OW], f16)

                    if folded:
                        r0 = 0
                        for nr in FOLD_ROWTILES:
                            ncols = nr * W
                            pa = ps_pool.tile([128, 1024], f32, tag="pt")
                            for c0, cn in _halves(ncols):
                                k = 0
                                for h in range(FH):
                                    for ch in range(2):
                                        nc.tensor.matmul(
                                            pa[:, c0 : c0 + cn],
                                            wt_sb[:, h * 2 + ch, :],
                                            xs_t[
                                                :,
                                                ch,
                                                (r0 + h) * W + c0 : (r0 + h) * W
                                                + c0
                                                + cn,
                                            ],
                                            start=(k == 0),
                                            stop=(k == 5),
                                        )
                                        k += 1
                            pav = pa[:, 0:ncols].rearrange(
                                "p (r w) -> p r w", w=W
                            )
                            dst = y3_t[:, r0 * OW : (r0 + nr) * OW].rearrange(
                                "p (r j) -> p r j", j=OW
                            )
                            nc.scalar.mul(
                                dst, pav[:, :, 0:OW], wsc_sb[:, 3:4]
                            )
                            nc.vector.scalar_tensor_tensor(
                                dst, pav[:, :, 1 : 1 + OW], wsc_sb[:, 4:5],
                                dst, op0=mult, op1=add,
                            )
                            nc.vector.scalar_tensor_tensor(
                                dst, pav[:, :, 2 : 2 + OW], wsc_sb[:, 5:6],
                                dst, op0=mult, op1=add,
                            )
                            r0 += nr
                    else:
                        y1_t = y1_pool.tile([128, NRI * W], f16)
                        for t0, tn in _tiles(NRI * W, 1024):
                            pa = ps_pool.tile([128, 1024], f32, tag="pt")
                            for c0, cn in _halves(tn):
                                for ch in range(2):
                                    nc.tensor.matmul(
                                        pa[:, c0 : c0 + cn],
                                        wt_sb[:, 6 + ch, :],
                                        xs_t[:, ch, t0 + c0 : t0 + c0 + cn],
                                        start=(ch == 0),
                                        stop=(ch == 1),
                                    )
                            nc.scalar.copy(y1_t[:, t0 : t0 + tn], pa[:, 0:tn])
                        # h-conv in fp16 on DVE (4x mul, 2x STT)
                        y2_t = y2_pool.tile([128, S * W], f16)
                        nc.vector.tensor_scalar_mul(
                            y2_t[:], y1_t[:, 0 : S * W], wsc_sb[:, 0:1]
                        )
                        nc.vector.scalar_tensor_tensor(
                            y2_t[:], y1_t[:, W : W + S * W], wsc_sb[:, 1:2],
                            y2_t[:], op0=mult, op1=add,
                        )
                        nc.vector.scalar_tensor_tensor(
                            y2_t[:], y1_t[:, 2 * W : 2 * W + S * W],
                            wsc_sb[:, 2:3], y2_t[:], op0=mult, op1=add,
                        )
                        # w-conv in fp16 on DVE
                        y2v = y2_t.rearrange("p (r w) -> p r w", w=W)
                        y3v = y3_t.rearrange("p (r j) -> p r j", j=OW)
                        nc.vector.tensor_scalar_mul(
                            y3v, y2v[:, :, 0:OW], wsc_sb[:, 3:4]
                        )
                        nc.vector.scalar_tensor_tensor(
                            y3v, y2v[:, :, 1 : 1 + OW], wsc_sb[:, 4:5], y3v,
                            op0=mult, op1=add,
                        )
                        nc.vector.scalar_tensor_tensor(
                            y3v, y2v[:, :, 2 : 2 + OW], wsc_sb[:, 5:6], y3v,
                            op0=mult, op1=add,
                        )

                    # stage D: projection R->F over flat col tiles of y3
                    ot = osb_pool.tile([128, 4, S * OW], f16)
                    for fc in range(4):
                        for c0, cn in _tiles(S * OW, 1024):
                            pd = ps_pool.tile([128, 1024], f32, tag="pt")
                            for h0, hn in _halves(cn):
                                nc.tensor.matmul(
                                    pd[:, h0 : h0 + hn],
                                    wt_sb[:, 8 + fc, :],
                                    y3_t[:, c0 + h0 : c0 + h0 + hn],
                                    start=True,
                                    stop=True,
                                )
                            d_copy(ot[:, fc, c0 : c0 + cn], pd[:, 0:cn])
                    nc.sync.dma_start(
                        od[b, :, i0 : i0 + S, :].rearrange(
                            "(c p) i j -> p c (i j)", p=128
                        ),
                        ot[:],
                    )

    nc.compile()
    return nc


def _get_nc():
    if "nc" not in _NC_CACHE:
        _NC_CACHE["nc"] = _build_nc()
    return _NC_CACHE["nc"]


def _prep_weights(factor0, factor1, factor2, factor3):
    wa = (factor3[None, :, :] * factor1[:, None, :]).reshape(FH, 2, 128, R)
    w3 = factor3.reshape(2, 128, R)
    w0 = factor0.reshape(4, 128, R).transpose(0, 2, 1)
    wt = np.concatenate(
        [wa.reshape(6, 128, R), w3, w0], axis=0
    ).astype(np.float16)
    wt = np.ascontiguousarray(wt)
    wsc = np.zeros((R, 8), dtype=np.float32)
    wsc[:, 0:3] = factor1.T
    wsc[:, 3:6] = factor2.T
    return wt, wsc


def _prep_x(x):
    return np.ascontiguousarray(x).astype(np.float16)


def _make_in_maps(x, factor0, factor1, factor2, factor3):
    wt, wsc = _prep_weights(factor0, factor1, factor2, factor3)
    x16 = _prep_x(x)
    return [
        {"x": x16[c * BLOC : (c + 1) * BLOC], "wt": wt, "wsc": wsc}
        for c in range(NCORES)
    ]


def kernel(x, factor0, factor1, factor2, factor3):
    from concourse import bass_utils

    x = np.asarray(x, dtype=np.float32)
    factor0 = np.asarray(factor0, dtype=np.float32)
    factor1 = np.asarray(factor1, dtype=np.float32)
    factor2 = np.asarray(factor2, dtype=np.float32)
    factor3 = np.asarray(factor3, dtype=np.float32)

    in_maps = _make_in_maps(x, factor0, factor1, factor2, factor3)
    nc = _get_nc()
    res = bass_utils.run_bass_kernel_spmd(nc, in_maps, list(range(NCORES)))
    out = np.concatenate(
        [res.results[c]["out"] for c in range(NCORES)], axis=0
    )
    return out.astype(np.float32)


# revision 5
# speedup vs baseline: 1.1635x; 1.0200x over previous
"""CP-decomposed conv (pointwise -> depthwise-h -> depthwise-w -> pointwise)
as a Bass/Tile kernel on 8 TRN2 NeuronCores.

Strategy (v2):
  - Data-parallel over batch: 32 images -> 4 per core, no collectives.
  - fp16 wire format for x and out (halves HBM bytes; HBM floor ~154us/core).
  - Per image, 2 row-strips of S=47 output rows (49 input rows with halo).
  - HYBRID h-conv schedule, tuned so PE / DVE / ACT all sit below the DMA
    roofline:
      * FOLDED strips: h-conv folded into the C->R pointwise matmul
        (6 accumulating fp16 matmuls per PSUM tile; 3x the PE work of a
        plain pointwise but zero DVE work), then the w-conv runs straight
        out of PSUM (ACT mul + 2 DVE STT per tile).
      * UNFOLDED strips: plain pointwise C->R (PE 3x cheaper), PSUM->SBUF
        cast copy on ACT, then h-conv and w-conv as fp16 SBUF ops on DVE
        where tensor_scalar runs at 4x and STT at 2x.
  - Final projection R->F: one 128x128 fp16 matmul per 512-col half of a
    2-bank PSUM tile; PSUM->SBUF cast copies split ACT/DVE by a weighted
    round-robin to balance the two engines.
  - Input DMAs on GpSimd SWDGE (idle engine), output DMAs on SP HWDGE;
    one ~2.4MB input DMA and one ~4.5MB output DMA per strip.
"""

import sys
import numpy as np

for _p in ("/opt/trn_rl_repo",):
    if _p not in sys.path:
        sys.path.insert(0, _p)

B, C, H, W = 32, 256, 96, 96
F, FH, FW, R = 512, 3, 3, 128
OH, OW = H - FH + 1, W - FW + 1  # 94, 94
NCORES = 8
BLOC = B // NCORES  # 4 images per core

S = 47                       # output rows per strip
STRIPS = [(0, S), (S, S)]    # per image
NRI = S + 2                  # input rows per strip (halo)

# fold schedule over the 8 (image, strip) pairs per core:
# 1 = h-conv folded into stage-A matmuls (PE-heavy),
# 0 = h-conv on DVE in fp16 (vector-heavy).
FOLD = [1, 1, 1, 1, 1, 1, 1, 1]

# fraction of stage-D PSUM->SBUF copy elements sent to DVE (rest on ACT)
DVE_COPY_FRAC = 0.31

# row tiles within a folded strip (rows per PSUM tile, <= 5 to fit 1 bank)
FOLD_ROWTILES = [5, 5, 5, 5, 5, 5, 5, 5, 5, 2]
# flat col tiles for a 2-bank (1024 fp32) PSUM tile
def _tiles(total, size):
    out, c0 = [], 0
    while c0 < total:
        t = min(size, total - c0)
        out.append((c0, t))
        c0 += t
    return out


def _halves(n):
    if n <= 512:
        return [(0, n)]
    return [(0, 512), (512, n - 512)]


_NC_CACHE = {}


def _build_nc():
    import concourse.bacc as bacc
    import concourse.mybir as mybir
    import concourse.tile as tile

    f32 = mybir.dt.float32
    f16 = mybir.dt.float16
    mult = mybir.AluOpType.mult
    add = mybir.AluOpType.add

    nc = bacc.Bacc("TRN2", target_bir_lowering=False, debug=True)

    xd = nc.dram_tensor("x", [BLOC, C, H, W], f16, kind="ExternalInput")
    # wt packs 12 [128,128] weight tiles:
    #   [0:6]  folded stage-A:  [h*2+ch, c', r] = f3[ch*128+c', r] * f1[h, r]
    #   [6:8]  plain stage-A:   [6+ch,   c', r] = f3[ch*128+c', r]
    #   [8:12] stage-D:         [8+fc,   r, f'] = f0[fc*128+f', r]
    wtd = nc.dram_tensor("wt", [12, 128, 128], f16, kind="ExternalInput")
    # wsc[r, 0:3] = f1[h, r]; wsc[r, 3:6] = f2[w, r]
    wscd = nc.dram_tensor("wsc", [R, 8], f32, kind="ExternalInput")
    od = nc.dram_tensor("out", [BLOC, F, OH, OW], f16, kind="ExternalOutput")

    with tile.TileContext(nc) as tc:
        with (
            tc.tile_pool(name="wpool", bufs=1) as wpool,
            tc.tile_pool(name="xs", bufs=3) as xs_pool,
            tc.tile_pool(name="y1p", bufs=2) as y1_pool,
            tc.tile_pool(name="y2p", bufs=2) as y2_pool,
            tc.tile_pool(name="y3p", bufs=2) as y3_pool,
            tc.tile_pool(name="osb", bufs=2) as osb_pool,
            tc.tile_pool(name="psa", bufs=4, space="PSUM") as psa_pool,
            tc.tile_pool(name="psd", bufs=4, space="PSUM") as psd_pool,
        ):
            wsc_sb = wpool.tile([128, 8], f32)
            nc.sync.dma_start(wsc_sb[:], wscd[:])
            wt_sb = wpool.tile([128, 12, 128], f16)
            nc.sync.dma_start(wt_sb[:], wtd.ap().rearrange("t p c -> p t c"))

            # weighted round-robin for stage-D copy engine assignment
            dve_credit = [0.0]

            def d_copy(dst, src):
                dve_credit[0] += DVE_COPY_FRAC
                if dve_credit[0] >= 1.0:
                    dve_credit[0] -= 1.0
                    nc.vector.tensor_copy(dst, src)
                else:
                    nc.scalar.copy(dst, src)

            ordinal = 0
            for b in range(BLOC):
                for i0, _S in STRIPS:
                    folded = FOLD[ordinal]
                    ordinal += 1

                    xs_t = xs_pool.tile([128, 2, NRI * W], f16)
                    nc.gpsimd.dma_start(
                        xs_t[:],
                        xd[b, :, i0 : i0 + NRI, :].rearrange(
                            "(c p) h w -> p c (h w)", p=128
                        ),
                    )

                    y3_t = y3_pool.tile([128, S * OW], f16)

                    if folded:
                        r0 = 0
                        for nr in FOLD_ROWTILES:
                            ncols = nr * W
                            pa = psa_pool.tile([128, 512], f32, tag="pa")
                            k = 0
                            for h in range(FH):
                                for ch in range(2):
                                    nc.tensor.matmul(
                                        pa[:, 0:ncols],
                                        wt_sb[:, h * 2 + ch, :],
                                        xs_t[
                                            :,
                                            ch,
                                            (r0 + h) * W : (r0 + h) * W + ncols,
                                        ],
                                        start=(k == 0),
                                        stop=(k == 5),
                                    )
                                    k += 1
                            pav = pa[:, 0:ncols].rearrange(
                                "p (r w) -> p r w", w=W
                            )
                            dst = y3_t[:, r0 * OW : (r0 + nr) * OW].rearrange(
                                "p (r j) -> p r j", j=OW
                            )
                            nc.scalar.mul(
                                dst, pav[:, :, 0:OW], wsc_sb[:, 3:4]
                            )
                            nc.vector.scalar_tensor_tensor(
                                dst, pav[:, :, 1 : 1 + OW], wsc_sb[:, 4:5],
                                dst, op0=mult, op1=add,
                            )
                            nc.vector.scalar_tensor_tensor(
                                dst, pav[:, :, 2 : 2 + OW], wsc_sb[:, 5:6],
                                dst, op0=mult, op1=add,
                            )
                            r0 += nr
                    else:
                        y1_t = y1_pool.tile([128, NRI * W], f16)
                        for t0, tn in _tiles(NRI * W, 512):
                            pa = psa_pool.tile([128, 512], f32, tag="pa")
                            for c0, cn in _halves(tn):
                                for ch in range(2):
                                    nc.tensor.matmul(
                                        pa[:, c0 : c0 + cn],
                                        wt_sb[:, 6 + ch, :],
                                        xs_t[:, ch, t0 + c0 : t0 + c0 + cn],
                                        start=(ch == 0),
                                        stop=(ch == 1),
                                    )
                            nc.scalar.copy(y1_t[:, t0 : t0 + tn], pa[:, 0:tn])
                        # h-conv in fp16 on DVE (4x mul, 2x STT)
                        y2_t = y2_pool.tile([128, S * W], f16)
                        nc.vector.tensor_scalar_mul(
                            y2_t[:], y1_t[:, 0 : S * W], wsc_sb[:, 0:1]
                        )
                        nc.vector.scalar_tensor_tensor(
                            y2_t[:], y1_t[:, W : W + S * W], wsc_sb[:, 1:2],
                            y2_t[:], op0=mult, op1=add,
                        )
                        nc.vector.scalar_tensor_tensor(
                            y2_t[:], y1_t[:, 2 * W : 2 * W + S * W],
                            wsc_sb[:, 2:3], y2_t[:], op0=mult, op1=add,
                        )
                        # w-conv in fp16 on DVE
                        y2v = y2_t.rearrange("p (r w) -> p r w", w=W)
                        y3v = y3_t.rearrange("p (r j) -> p r j", j=OW)
                        nc.vector.tensor_scalar_mul(
                            y3v, y2v[:, :, 0:OW], wsc_sb[:, 3:4]
                        )
                        nc.vector.scalar_tensor_tensor(
                            y3v, y2v[:, :, 1 : 1 + OW], wsc_sb[:, 4:5], y3v,
                            op0=mult, op1=add,
                        )
                        nc.vector.scalar_tensor_tensor(
                            y3v, y2v[:, :, 2 : 2 + OW], wsc_sb[:, 5:6], y3v,
                            op0=mult, op1=add,
                        )

                    # stage D: projection R->F over flat col tiles of y3
                    ot = osb_pool.tile([128, 4, S * OW], f16)
                    for fc in range(4):
                        for c0, cn in _tiles(S * OW, 512):
                            pd = psd_pool.tile([128, 512], f32, tag="pd")
                            nc.tensor.matmul(
                                pd[:, 0:cn],
                                wt_sb[:, 8 + fc, :],
                                y3_t[:, c0 : c0 + cn],
                                start=True,
                                stop=True,
                            )
                            d_copy(ot[:, fc, c0 : c0 + cn], pd[:, 0:cn])
                    nc.sync.dma_start(
                        od[b, :, i0 : i0 + S, :].rearrange(
                            "(c p) i j -> p c (i j)", p=128
                        ),
                        ot[:],
                    )

    nc.compile()
    return nc


def _get_nc():
    if "nc" not in _NC_CACHE:
        _NC_CACHE["nc"] = _build_nc()
    return _NC_CACHE["nc"]


def _prep_weights(factor0, factor1, factor2, factor3):
    wa = (factor3[None, :, :] * factor1[:, None, :]).reshape(FH, 2, 128, R)
    w3 = factor3.reshape(2, 128, R)
    w0 = factor0.reshape(4, 128, R).transpose(0, 2, 1)
    wt = np.concatenate(
        [wa.reshape(6, 128, R), w3, w0], axis=0
    ).astype(np.float16)
    wt = np.ascontiguousarray(wt)
    wsc = np.zeros((R, 8), dtype=np.float32)
    wsc[:, 0:3] = factor1.T
    wsc[:, 3:6] = factor2.T
    return wt, wsc


def _prep_x(x):
    return np.ascontiguousarray(x).astype(np.float16)


def _make_in_maps(x, factor0, factor1, factor2, factor3):
    wt, wsc = _prep_weights(factor0, factor1, factor2, factor3)
    x16 = _prep_x(x)
    return [
        {"x": x16[c * BLOC : (c + 1) * BLOC], "wt": wt, "wsc": wsc}
        for c in range(NCORES)
    ]


def kernel(x, factor0, factor1, factor2, factor3):
    from concourse import bass_utils

    x = np.asarray(x, dtype=np.float32)
    factor0 = np.asarray(factor0, dtype=np.float32)
    factor1 = np.asarray(factor1, dtype=np.float32)
    factor2 = np.asarray(factor2, dtype=np.float32)
    factor3 = np.asarray(factor3, dtype=np.float32)

    in_maps = _make_in_maps(x, factor0, factor1, factor2, factor3)
    nc = _get_nc()
    res = bass_utils.run_bass_kernel_spmd(nc, in_maps, list(range(NCORES)))
    out = np.concatenate(
        [res.results[c]["out"] for c in range(NCORES)], axis=0
    )
    return out.astype(np.float32)


# revision 6
# speedup vs baseline: 1.2808x; 1.1008x over previous
"""CP-decomposed conv (pointwise -> depthwise-h -> depthwise-w -> pointwise)
as a Bass/Tile kernel on 8 TRN2 NeuronCores.

Strategy (v2):
  - Data-parallel over batch: 32 images -> 4 per core, no collectives.
  - fp16 wire format for x and out (halves HBM bytes; HBM floor ~154us/core).
  - Per image, 2 row-strips of S=47 output rows (49 input rows with halo).
  - HYBRID h-conv schedule, tuned so PE / DVE / ACT all sit below the DMA
    roofline:
      * FOLDED strips: h-conv folded into the C->R pointwise matmul
        (6 accumulating fp16 matmuls per PSUM tile; 3x the PE work of a
        plain pointwise but zero DVE work), then the w-conv runs straight
        out of PSUM (ACT mul + 2 DVE STT per tile).
      * UNFOLDED strips: plain pointwise C->R (PE 3x cheaper), PSUM->SBUF
        cast copy on ACT, then h-conv and w-conv as fp16 SBUF ops on DVE
        where tensor_scalar runs at 4x and STT at 2x.
  - Final projection R->F: one 128x128 fp16 matmul per 512-col half of a
    2-bank PSUM tile; PSUM->SBUF cast copies split ACT/DVE by a weighted
    round-robin to balance the two engines.
  - Input DMAs on GpSimd SWDGE (idle engine), output DMAs on SP HWDGE;
    one ~2.4MB input DMA and one ~4.5MB output DMA per strip.
"""

import sys
import numpy as np

for _p in ("/opt/trn_rl_repo",):
    if _p not in sys.path:
        sys.path.insert(0, _p)

B, C, H, W = 32, 256, 96, 96
F, FH, FW, R = 512, 3, 3, 128
OH, OW = H - FH + 1, W - FW + 1  # 94, 94
NCORES = 8
BLOC = B // NCORES  # 4 images per core

S = 47                       # output rows per strip
STRIPS = [(0, S), (S, S)]    # per image
NRI = S + 2                  # input rows per strip (halo)

# fold schedule over the 8 (image, strip) pairs per core:
# 1 = h-conv folded into stage-A matmuls (PE-heavy),
# 0 = h-conv on DVE in fp16 (vector-heavy).
FOLD = [1, 1, 1, 1, 1, 1, 1, 1]

# fraction of stage-D PSUM->SBUF copy elements sent to DVE (rest on ACT)
DVE_COPY_FRAC = 0.31

# row tiles within a folded strip (rows per PSUM tile, <= 5 to fit 1 bank)
FOLD_ROWTILES = [5, 5, 5, 5, 5, 5, 5, 5, 5, 2]
# flat col tiles for a 2-bank (1024 fp32) PSUM tile
def _tiles(total, size):
    out, c0 = [], 0
    while c0 < total:
        t = min(size, total - c0)
        out.append((c0, t))
        c0 += t
    return out


def _halves(n):
    if n <= 512:
        return [(0, n)]
    return [(0, 512), (512, n - 512)]


_NC_CACHE = {}


def _build_nc():
    import concourse.bacc as bacc
    import concourse.mybir as mybir
    import concourse.tile as tile

    f32 = mybir.dt.float32
    f16 = mybir.dt.float16
    mult = mybir.AluOpType.mult
    add = mybir.AluOpType.add

    nc = bacc.Bacc("TRN2", target_bir_lowering=False, debug=True)

    xd = nc.dram_tensor("x", [BLOC, C, H, W], f16, kind="ExternalInput")
    # wt packs 12 [128,128] weight tiles:
    #   [0:6]  folded stage-A:  [h*2+ch, c', r] = f3[ch*128+c', r] * f1[h, r]
    #   [6:8]  plain stage-A:   [6+ch,   c', r] = f3[ch*128+c', r]
    #   [8:12] stage-D:         [8+fc,   r, f'] = f0[fc*128+f', r]
    wtd = nc.dram_tensor("wt", [128, 12, 128], f16, kind="ExternalInput")
    # wsc[r, 0:3] = f1[h, r]; wsc[r, 3:6] = f2[w, r]
    wscd = nc.dram_tensor("wsc", [R, 8], f32, kind="ExternalInput")
    od = nc.dram_tensor("out", [BLOC, F, OH, OW], f16, kind="ExternalOutput")

    with tile.TileContext(nc) as tc:
        with (
            tc.tile_pool(name="wpool", bufs=1) as wpool,
            tc.tile_pool(name="xs", bufs=4) as xs_pool,
            tc.tile_pool(name="y1p", bufs=2) as y1_pool,
            tc.tile_pool(name="y2p", bufs=2) as y2_pool,
            tc.tile_pool(name="y3p", bufs=2) as y3_pool,
            tc.tile_pool(name="osb", bufs=2) as osb_pool,
            tc.tile_pool(name="psa", bufs=4, space="PSUM") as psa_pool,
            tc.tile_pool(name="psd", bufs=4, space="PSUM") as psd_pool,
        ):
            wsc_sb = wpool.tile([128, 8], f32)
            nc.sync.dma_start(wsc_sb[:], wscd[:])
            wt_sb = wpool.tile([128, 12, 128], f16)
            nc.sync.dma_start(wt_sb[:], wtd[:])

            # weighted round-robin for stage-D copy engine assignment
            dve_credit = [0.0]

            def d_copy(dst, src):
                dve_credit[0] += DVE_COPY_FRAC
                if dve_credit[0] >= 1.0:
                    dve_credit[0] -= 1.0
                    nc.vector.tensor_copy(dst, src)
                else:
                    nc.scalar.copy(dst, src)

            ordinal = 0
            for b in range(BLOC):
                for i0, _S in STRIPS:
                    folded = FOLD[ordinal]
                    ordinal += 1

                    xs_t = xs_pool.tile([128, 2, NRI * W], f16)
                    for ch in range(2):
                        nc.gpsimd.dma_start(
                            xs_t[:, ch, :],
                            xd[b, ch * 128 : (ch + 1) * 128, i0 : i0 + NRI, :],
                        )

                    y3_t = y3_pool.tile([128, S * OW], f16)

                    if folded:
                        r0 = 0
                        for nr in FOLD_ROWTILES:
                            ncols = nr * W
                            pa = psa_pool.tile([128, 512], f32, tag="pa")
                            k = 0
                            for ch in range(2):
                                for h in range(FH):
                                    nc.tensor.matmul(
                                        pa[:, 0:ncols],
                                        wt_sb[:, h * 2 + ch, :],
                                        xs_t[
                                            :,
                                            ch,
                                            (r0 + h) * W : (r0 + h) * W + ncols,
                                        ],
                                        start=(k == 0),
                                        stop=(k == 5),
                                    )
                                    k += 1
                            pav = pa[:, 0:ncols].rearrange(
                                "p (r w) -> p r w", w=W
                            )
                            dst = y3_t[:, r0 * OW : (r0 + nr) * OW].rearrange(
                                "p (r j) -> p r j", j=OW
                            )
                            nc.scalar.mul(
                                dst, pav[:, :, 0:OW], wsc_sb[:, 3:4]
                            )
                            nc.vector.scalar_tensor_tensor(
                                dst, pav[:, :, 1 : 1 + OW], wsc_sb[:, 4:5],
                                dst, op0=mult, op1=add,
                            )
                            nc.vector.scalar_tensor_tensor(
                                dst, pav[:, :, 2 : 2 + OW], wsc_sb[:, 5:6],
                                dst, op0=mult, op1=add,
                            )
                            r0 += nr
                    else:
                        y1_t = y1_pool.tile([128, NRI * W], f16)
                        for t0, tn in _tiles(NRI * W, 512):
                            pa = psa_pool.tile([128, 512], f32, tag="pa")
                            for c0, cn in _halves(tn):
                                for ch in range(2):
                                    nc.tensor.matmul(
                                        pa[:, c0 : c0 + cn],
                                        wt_sb[:, 6 + ch, :],
                                        xs_t[:, ch, t0 + c0 : t0 + c0 + cn],
                                        start=(ch == 0),
                                        stop=(ch == 1),
                                    )
                            nc.scalar.copy(y1_t[:, t0 : t0 + tn], pa[:, 0:tn])
                        # h-conv in fp16 on DVE (4x mul, 2x STT)
                        y2_t = y2_pool.tile([128, S * W], f16)
                        nc.vector.tensor_scalar_mul(
                            y2_t[:], y1_t[:, 0 : S * W], wsc_sb[:, 0:1]
                        )
                        nc.vector.scalar_tensor_tensor(
                            y2_t[:], y1_t[:, W : W + S * W], wsc_sb[:, 1:2],
                            y2_t[:], op0=mult, op1=add,
                        )
                        nc.vector.scalar_tensor_tensor(
                            y2_t[:], y1_t[:, 2 * W : 2 * W + S * W],
                            wsc_sb[:, 2:3], y2_t[:], op0=mult, op1=add,
                        )
                        # w-conv in fp16 on DVE
                        y2v = y2_t.rearrange("p (r w) -> p r w", w=W)
                        y3v = y3_t.rearrange("p (r j) -> p r j", j=OW)
                        nc.vector.tensor_scalar_mul(
                            y3v, y2v[:, :, 0:OW], wsc_sb[:, 3:4]
                        )
                        nc.vector.scalar_tensor_tensor(
                            y3v, y2v[:, :, 1 : 1 + OW], wsc_sb[:, 4:5], y3v,
                            op0=mult, op1=add,
                        )
                        nc.vector.scalar_tensor_tensor(
                            y3v, y2v[:, :, 2 : 2 + OW], wsc_sb[:, 5:6], y3v,
                            op0=mult, op1=add,
                        )

                    # stage D: projection R->F over flat col tiles of y3
                    ot = osb_pool.tile([128, 4, S * OW], f16)
                    for fc in range(4):
                        for c0, cn in _tiles(S * OW, 512):
                            pd = psd_pool.tile([128, 512], f32, tag="pd")
                            nc.tensor.matmul(
                                pd[:, 0:cn],
                                wt_sb[:, 8 + fc, :],
                                y3_t[:, c0 : c0 + cn],
                                start=True,
                                stop=True,
                            )
                            d_copy(ot[:, fc, c0 : c0 + cn], pd[:, 0:cn])
                        nc.sync.dma_start(
                            od[b, fc * 128 : (fc + 1) * 128, i0 : i0 + S, :],
                            ot[:, fc, :],
                        )

    nc.compile()
    return nc


def _get_nc():
    if "nc" not in _NC_CACHE:
        _NC_CACHE["nc"] = _build_nc()
    return _NC_CACHE["nc"]


def _prep_weights(factor0, factor1, factor2, factor3):
    wa = (factor3[None, :, :] * factor1[:, None, :]).reshape(FH, 2, 128, R)
    w3 = factor3.reshape(2, 128, R)
    w0 = factor0.reshape(4, 128, R).transpose(0, 2, 1)
    wt = np.concatenate(
        [wa.reshape(6, 128, R), w3, w0], axis=0
    ).astype(np.float16)
    wt = np.ascontiguousarray(wt.transpose(1, 0, 2))
    wsc = np.zeros((R, 8), dtype=np.float32)
    wsc[:, 0:3] = factor1.T
    wsc[:, 3:6] = factor2.T
    return wt, wsc


def _prep_x(x):
    return np.ascontiguousarray(x).astype(np.float16)


def _make_in_maps(x, factor0, factor1, factor2, factor3):
    wt, wsc = _prep_weights(factor0, factor1, factor2, factor3)
    x16 = _prep_x(x)
    return [
        {"x": x16[c * BLOC : (c + 1) * BLOC], "wt": wt, "wsc": wsc}
        for c in range(NCORES)
    ]


def kernel(x, factor0, factor1, factor2, factor3):
    from concourse import bass_utils

    x = np.asarray(x, dtype=np.float32)
    factor0 = np.asarray(factor0, dtype=np.float32)
    factor1 = np.asarray(factor1, dtype=np.float32)
    factor2 = np.asarray(factor2, dtype=np.float32)
    factor3 = np.asarray(factor3, dtype=np.float32)

    in_maps = _make_in_maps(x, factor0, factor1, factor2, factor3)
    nc = _get_nc()
    res = bass_utils.run_bass_kernel_spmd(nc, in_maps, list(range(NCORES)))
    out = np.concatenate(
        [res.results[c]["out"] for c in range(NCORES)], axis=0
    )
    return out.astype(np.float32)


# revision 7
# speedup vs baseline: 1.3998x; 1.0929x over previous
"""CP-decomposed conv (pointwise -> depthwise-h -> depthwise-w -> pointwise)
as a Bass/Tile kernel on 8 TRN2 NeuronCores.

Strategy (v2):
  - Data-parallel over batch: 32 images -> 4 per core, no collectives.
  - fp16 wire format for x and out (halves HBM bytes; HBM floor ~154us/core).
  - Per image, 2 row-strips of S=47 output rows (49 input rows with halo).
  - HYBRID h-conv schedule, tuned so PE / DVE / ACT all sit below the DMA
    roofline:
      * FOLDED strips: h-conv folded into the C->R pointwise matmul
        (6 accumulating fp16 matmuls per PSUM tile; 3x the PE work of a
        plain pointwise but zero DVE work), then the w-conv runs straight
        out of PSUM (ACT mul + 2 DVE STT per tile).
      * UNFOLDED strips: plain pointwise C->R (PE 3x cheaper), PSUM->SBUF
        cast copy on ACT, then h-conv and w-conv as fp16 SBUF ops on DVE
        where tensor_scalar runs at 4x and STT at 2x.
  - Final projection R->F: one 128x128 fp16 matmul per 512-col half of a
    2-bank PSUM tile; PSUM->SBUF cast copies split ACT/DVE by a weighted
    round-robin to balance the two engines.
  - Input DMAs on GpSimd SWDGE (idle engine), output DMAs on SP HWDGE;
    one ~2.4MB input DMA and one ~4.5MB output DMA per strip.
"""

import sys
import numpy as np

for _p in ("/opt/trn_rl_repo",):
    if _p not in sys.path:
        sys.path.insert(0, _p)

B, C, H, W = 32, 256, 96, 96
F, FH, FW, R = 512, 3, 3, 128
OH, OW = H - FH + 1, W - FW + 1  # 94, 94
NCORES = 8
BLOC = B // NCORES  # 4 images per core

S = 47                       # output rows per strip
STRIPS = [(0, S), (S, S)]    # per image
NRI = S + 2                  # input rows per strip (halo)

# fold schedule over the 8 (image, strip) pairs per core:
# 1 = h-conv folded into stage-A matmuls (PE-heavy),
# 0 = h-conv on DVE in fp16 (vector-heavy).
FOLD = [1, 1, 1, 1, 1, 1, 1, 1]

# fraction of stage-D PSUM->SBUF copy elements sent to DVE (rest on ACT)
DVE_COPY_FRAC = 0.31

# row tiles within a folded strip (rows per PSUM tile, <= 5 to fit 1 bank)
FOLD_ROWTILES = [5, 5, 5, 5, 5, 5, 5, 5, 5, 2]
# flat col tiles for a 2-bank (1024 fp32) PSUM tile
def _tiles(total, size):
    out, c0 = [], 0
    while c0 < total:
        t = min(size, total - c0)
        out.append((c0, t))
        c0 += t
    return out


def _halves(n):
    if n <= 512:
        return [(0, n)]
    return [(0, 512), (512, n - 512)]


_NC_CACHE = {}


def _build_nc():
    import concourse.bacc as bacc
    import concourse.mybir as mybir
    import concourse.tile as tile

    f32 = mybir.dt.float32
    f16 = mybir.dt.float16
    mult = mybir.AluOpType.mult
    add = mybir.AluOpType.add

    nc = bacc.Bacc("TRN2", target_bir_lowering=False, debug=True)

    xd = nc.dram_tensor("x", [BLOC, C, H, W], f16, kind="ExternalInput")
    # wt packs 12 [128,128] weight tiles:
    #   [0:6]  folded stage-A:  [h*2+ch, c', r] = f3[ch*128+c', r] * f1[h, r]
    #   [6:8]  plain stage-A:   [6+ch,   c', r] = f3[ch*128+c', r]
    #   [8:12] stage-D:         [8+fc,   r, f'] = f0[fc*128+f', r]
    wtd = nc.dram_tensor("wt", [128, 12, 128], f16, kind="ExternalInput")
    # wsc[r, 0:3] = f1[h, r]; wsc[r, 3:6] = f2[w, r]
    wscd = nc.dram_tensor("wsc", [R, 8], f32, kind="ExternalInput")
    od = nc.dram_tensor("out", [BLOC, F, OH, OW], f16, kind="ExternalOutput")

    with tile.TileContext(nc) as tc:
        with (
            tc.tile_pool(name="wpool", bufs=1) as wpool,
            tc.tile_pool(name="xs", bufs=4) as xs_pool,
            tc.tile_pool(name="y1p", bufs=2) as y1_pool,
            tc.tile_pool(name="y2p", bufs=2) as y2_pool,
            tc.tile_pool(name="y3p", bufs=2) as y3_pool,
            tc.tile_pool(name="osb", bufs=2) as osb_pool,
            tc.tile_pool(name="psa", bufs=4, space="PSUM") as psa_pool,
            tc.tile_pool(name="psd", bufs=4, space="PSUM") as psd_pool,
        ):
            wsc_sb = wpool.tile([128, 8], f32)
            nc.sync.dma_start(wsc_sb[:], wscd[:])
            wt_sb = wpool.tile([128, 12, 128], f16)
            nc.sync.dma_start(wt_sb[:], wtd[:])

            # weighted round-robin for stage-D copy engine assignment
            dve_credit = [0.0]

            def d_copy(dst, src):
                dve_credit[0] += DVE_COPY_FRAC
                if dve_credit[0] >= 1.0:
                    dve_credit[0] -= 1.0
                    nc.vector.tensor_copy(dst, src)
                else:
                    nc.scalar.copy(dst, src)

            ordinal = 0
            for b in range(BLOC):
                for i0, _S in STRIPS:
                    folded = FOLD[ordinal]
                    ordinal += 1

                    xs_t = xs_pool.tile([128, 2, NRI * W], f16)
                    for ch in range(2):
                        nc.gpsimd.dma_start(
                            xs_t[:, ch, :],
                            xd[b, ch * 128 : (ch + 1) * 128, i0 : i0 + NRI, :],
                        )

                    y3_t = y3_pool.tile([128, S * OW], f16)

                    if folded:
                        # D col-tiles interleaved into the A loop as soon as
                        # their y3 range has been emitted (keeps every engine
                        # queue aligned with dataflow; no phase boundaries)
                        ot = osb_pool.tile([128, 4, S * OW], f16)
                        d_tiles = [
                            (c0, cn, fc)
                            for c0, cn in _tiles(S * OW, 512)
                            for fc in range(4)
                        ]
                        d_emitted = 0

                        def emit_d(limit_cols):
                            nonlocal d_emitted
                            while d_emitted < len(d_tiles):
                                c0, cn, fc = d_tiles[d_emitted]
                                if c0 + cn > limit_cols:
                                    break
                                pd = psd_pool.tile(
                                    [128, 512], f32, tag="pd", name="pd"
                                )
                                nc.tensor.matmul(
                                    pd[:, 0:cn],
                                    wt_sb[:, 8 + fc, :],
                                    y3_t[:, c0 : c0 + cn],
                                    start=True,
                                    stop=True,
                                )
                                d_copy(ot[:, fc, c0 : c0 + cn], pd[:, 0:cn])
                                d_emitted += 1

                        r0 = 0
                        for nr in FOLD_ROWTILES:
                            ncols = nr * W
                            pa = psa_pool.tile([128, 512], f32, tag="pa")
                            k = 0
                            for ch in range(2):
                                for h in range(FH):
                                    nc.tensor.matmul(
                                        pa[:, 0:ncols],
                                        wt_sb[:, h * 2 + ch, :],
                                        xs_t[
                                            :,
                                            ch,
                                            (r0 + h) * W : (r0 + h) * W + ncols,
                                        ],
                                        start=(k == 0),
                                        stop=(k == 5),
                                    )
                                    k += 1
                            pav = pa[:, 0:ncols].rearrange(
                                "p (r w) -> p r w", w=W
                            )
                            dst = y3_t[:, r0 * OW : (r0 + nr) * OW].rearrange(
                                "p (r j) -> p r j", j=OW
                            )
                            nc.scalar.mul(
                                dst, pav[:, :, 0:OW], wsc_sb[:, 3:4]
                            )
                            nc.vector.scalar_tensor_tensor(
                                dst, pav[:, :, 1 : 1 + OW], wsc_sb[:, 4:5],
                                dst, op0=mult, op1=add,
                            )
                            nc.vector.scalar_tensor_tensor(
                                dst, pav[:, :, 2 : 2 + OW], wsc_sb[:, 5:6],
                                dst, op0=mult, op1=add,
                            )
                            r0 += nr
                            emit_d(r0 * OW)
                        emit_d(S * OW + 1)
                        for fc in range(4):
                            nc.sync.dma_start(
                                od[b, fc * 128 : (fc + 1) * 128, i0 : i0 + S, :],
                                ot[:, fc, :],
                            )
                    else:
                        y1_t = y1_pool.tile([128, NRI * W], f16)
                        for t0, tn in _tiles(NRI * W, 512):
                            pa = psa_pool.tile([128, 512], f32, tag="pa")
                            for c0, cn in _halves(tn):
                                for ch in range(2):
                                    nc.tensor.matmul(
                                        pa[:, c0 : c0 + cn],
                                        wt_sb[:, 6 + ch, :],
                                        xs_t[:, ch, t0 + c0 : t0 + c0 + cn],
                                        start=(ch == 0),
                                        stop=(ch == 1),
                                    )
                            nc.scalar.copy(y1_t[:, t0 : t0 + tn], pa[:, 0:tn])
                        # h-conv in fp16 on DVE (4x mul, 2x STT)
                        y2_t = y2_pool.tile([128, S * W], f16)
                        nc.vector.tensor_scalar_mul(
                            y2_t[:], y1_t[:, 0 : S * W], wsc_sb[:, 0:1]
                        )
                        nc.vector.scalar_tensor_tensor(
                            y2_t[:], y1_t[:, W : W + S * W], wsc_sb[:, 1:2],
                            y2_t[:], op0=mult, op1=add,
                        )
                        nc.vector.scalar_tensor_tensor(
                            y2_t[:], y1_t[:, 2 * W : 2 * W + S * W],
                            wsc_sb[:, 2:3], y2_t[:], op0=mult, op1=add,
                        )
                        # w-conv in fp16 on DVE
                        y2v = y2_t.rearrange("p (r w) -> p r w", w=W)
                        y3v = y3_t.rearrange("p (r j) -> p r j", j=OW)
                        nc.vector.tensor_scalar_mul(
                            y3v, y2v[:, :, 0:OW], wsc_sb[:, 3:4]
                        )
                        nc.vector.scalar_tensor_tensor(
                            y3v, y2v[:, :, 1 : 1 + OW], wsc_sb[:, 4:5], y3v,
                            op0=mult, op1=add,
                        )
                        nc.vector.scalar_tensor_tensor(
                            y3v, y2v[:, :, 2 : 2 + OW], wsc_sb[:, 5:6], y3v,
                            op0=mult, op1=add,
                        )

                    if not folded:
                        # stage D for the unfolded path
                        ot = osb_pool.tile([128, 4, S * OW], f16)
                        for fc in range(4):
                            for c0, cn in _tiles(S * OW, 512):
                                pd = psd_pool.tile([128, 512], f32, tag="pd")
                                nc.tensor.matmul(
                                    pd[:, 0:cn],
                                    wt_sb[:, 8 + fc, :],
                                    y3_t[:, c0 : c0 + cn],
                                    start=True,
                                    stop=True,
                                )
                                d_copy(ot[:, fc, c0 : c0 + cn], pd[:, 0:cn])
                            nc.sync.dma_start(
                                od[b, fc * 128 : (fc + 1) * 128, i0 : i0 + S, :],
                                ot[:, fc, :],
                            )

    nc.compile()
    return nc


def _get_nc():
    if "nc" not in _NC_CACHE:
        _NC_CACHE["nc"] = _build_nc()
    return _NC_CACHE["nc"]


def _prep_weights(factor0, factor1, factor2, factor3):
    wa = (factor3[None, :, :] * factor1[:, None, :]).reshape(FH, 2, 128, R)
    w3 = factor3.reshape(2, 128, R)
    w0 = factor0.reshape(4, 128, R).transpose(0, 2, 1)
    wt = np.concatenate(
        [wa.reshape(6, 128, R), w3, w0], axis=0
    ).astype(np.float16)
    wt = np.ascontiguousarray(wt.transpose(1, 0, 2))
    wsc = np.zeros((R, 8), dtype=np.float32)
    wsc[:, 0:3] = factor1.T
    wsc[:, 3:6] = factor2.T
    return wt, wsc


def _prep_x(x):
    return np.ascontiguousarray(x).astype(np.float16)


def _make_in_maps(x, factor0, factor1, factor2, factor3):
    wt, wsc = _prep_weights(factor0, factor1, factor2, factor3)
    x16 = _prep_x(x)
    return [
        {"x": x16[c * BLOC : (c + 1) * BLOC], "wt": wt, "wsc": wsc}
        for c in range(NCORES)
    ]


def kernel(x, factor0, factor1, factor2, factor3):
    from concourse import bass_utils

    x = np.asarray(x, dtype=np.float32)
    factor0 = np.asarray(factor0, dtype=np.float32)
    factor1 = np.asarray(factor1, dtype=np.float32)
    factor2 = np.asarray(factor2, dtype=np.float32)
    factor3 = np.asarray(factor3, dtype=np.float32)

    in_maps = _make_in_maps(x, factor0, factor1, factor2, factor3)
    nc = _get_nc()
    res = bass_utils.run_bass_kernel_spmd(nc, in_maps, list(range(NCORES)))
    out = np.concatenate(
        [res.results[c]["out"] for c in range(NCORES)], axis=0
    )
    return out.astype(np.float32)


# revision 8
# speedup vs baseline: 1.5766x; 1.1263x over previous
"""CP-decomposed conv (pointwise -> depthwise-h -> depthwise-w -> pointwise)
as a Bass/Tile kernel on 8 TRN2 NeuronCores.

Strategy (v6):
  - Data-parallel over batch: 32 images -> 4 per core, no collectives.
  - fp16 wire format for x and out (halves HBM bytes; HBM floor ~155us/core).
  - Per image, 2 row-strips of S=47 output rows (49 input rows with halo).
  - h-conv folded into the C->R pointwise matmul: 6 accumulating fp16
    matmuls per 1-bank PSUM tile (5 output rows x 96 cols). The w-conv
    runs straight out of PSUM (ACT mul + 2 DVE STT per tile), since
    scalar_tensor_tensor has no 2x mode on TRN2 anyway.
  - Final projection R->F: one 128x128 fp16 matmul per 512-col 1-bank
    PSUM tile; PSUM->SBUF cast copies split ACT/DVE by weighted
    round-robin (the two engines are the real bottleneck besides DMA).
  - Software-pipelined emission: D col-tiles are interleaved into the A
    row-tile loop one tile behind their y3 producers, and each strip's
    tail D-tiles are flushed after the NEXT strip's first A-tiles so the
    PE queue never head-of-line blocks on a vector-chain tail.
  - Output DMAs split per fc and row-half so the store stream starts
    early; first strip's input DMA split so the first matmul starts ~8us
    after kernel start. Inputs on GpSimd SWDGE, outputs on SP HWDGE.
"""

import sys
import numpy as np

for _p in ("/opt/trn_rl_repo",):
    if _p not in sys.path:
        sys.path.insert(0, _p)

B, C, H, W = 32, 256, 96, 96
F, FH, FW, R = 512, 3, 3, 128
OH, OW = H - FH + 1, W - FW + 1  # 94, 94
NCORES = 8
BLOC = B // NCORES  # 4 images per core

S = 47                       # output rows per strip
STRIPS = [(0, S), (S, S)]    # per image
NRI = S + 2                  # input rows per strip (halo)

# fraction of stage-D PSUM->SBUF copy elements sent to DVE (rest on ACT)
DVE_COPY_FRAC = 0.34

# rows per stage-A PSUM tile (5 rows x 96 cols = 480 fp32 <= 1 bank)
FOLD_ROWTILES = [5, 5, 5, 5, 5, 5, 5, 5, 5, 2]
# output DMA row split: issue rows [0:OUT_R1) as soon as their copies land
OUT_R1 = 21  # 21*94 = 1974 cols, covered once the c0=1536 tile is copied


def _tiles(total, size):
    out, c0 = [], 0
    while c0 < total:
        t = min(size, total - c0)
        out.append((c0, t))
        c0 += t
    return out


_NC_CACHE = {}


def _build_nc():
    import concourse.bacc as bacc
    import concourse.mybir as mybir
    import concourse.tile as tile

    f32 = mybir.dt.float32
    f16 = mybir.dt.float16
    mult = mybir.AluOpType.mult
    add = mybir.AluOpType.add

    nc = bacc.Bacc("TRN2", target_bir_lowering=False, debug=True)

    xd = nc.dram_tensor("x", [BLOC, C, H, W], f16, kind="ExternalInput")
    # wt packs 12 [128,128] weight tiles, partition-major in DRAM:
    #   [0:6]  folded stage-A:  [c', h*2+ch, r] = f3[ch*128+c', r] * f1[h, r]
    #   [6:8]  (unused in this variant; plain f3 tiles)
    #   [8:12] stage-D:         [r, 8+fc, f'] = f0[fc*128+f', r]
    wtd = nc.dram_tensor("wt", [128, 12, 128], f16, kind="ExternalInput")
    # wsc[r, 0:3] = f1[h, r]; wsc[r, 3:6] = f2[w, r]
    wscd = nc.dram_tensor("wsc", [R, 8], f32, kind="ExternalInput")
    od = nc.dram_tensor("out", [BLOC, F, OH, OW], f16, kind="ExternalOutput")

    d_cols = _tiles(S * OW, 512)  # [(0,512)..(4096,322)]

    with tile.TileContext(nc) as tc:
        with (
            tc.tile_pool(name="wpool", bufs=1) as wpool,
            tc.tile_pool(name="xs", bufs=4) as xs_pool,
            tc.tile_pool(name="y3p", bufs=2) as y3_pool,
            tc.tile_pool(name="osb", bufs=2) as osb_pool,
            tc.tile_pool(name="psa", bufs=4, space="PSUM") as psa_pool,
            tc.tile_pool(name="psd", bufs=4, space="PSUM") as psd_pool,
        ):
            wsc_sb = wpool.tile([128, 8], f32)
            nc.sync.dma_start(wsc_sb[:], wscd[:])
            wt_sb = wpool.tile([128, 12, 128], f16)
            nc.sync.dma_start(wt_sb[:], wtd[:])

            dve_credit = [0.0]

            def d_copy(dst, src):
                dve_credit[0] += DVE_COPY_FRAC
                if dve_credit[0] >= 1.0:
                    dve_credit[0] -= 1.0
                    nc.vector.tensor_copy(dst, src)
                else:
                    nc.scalar.copy(dst, src)

            class StripD:
                """Emits stage-D tiles for one strip, interleaved into the
                A-tile loop; issues the per-fc output DMAs at the right
                copy boundaries."""

                def __init__(self, b, i0, y3_t, ot):
                    self.b = b
                    self.i0 = i0
                    self.y3_t = y3_t
                    self.ot = ot
                    self.tiles = [
                        (c0, cn, fc) for c0, cn in d_cols for fc in range(4)
                    ]
                    self.i = 0

                def emit(self, limit_cols):
                    while self.i < len(self.tiles):
                        c0, cn, fc = self.tiles[self.i]
                        if c0 + cn > limit_cols:
                            break
                        pd = psd_pool.tile([128, 512], f32, tag="pd", name="pd")
                        nc.tensor.matmul(
                            pd[:, 0:cn],
                            wt_sb[:, 8 + fc, :],
                            self.y3_t[:, c0 : c0 + cn],
                            start=True,
                            stop=True,
                        )
                        d_copy(self.ot[:, fc, c0 : c0 + cn], pd[:, 0:cn])
                        self.i += 1
                        if c0 + cn == 2048:
                            nc.sync.dma_start(
                                od[
                                    self.b,
                                    fc * 128 : (fc + 1) * 128,
                                    self.i0 : self.i0 + OUT_R1,
                                    :,
                                ],
                                self.ot[:, fc, 0 : OUT_R1 * OW],
                            )
                        elif c0 + cn == S * OW:
                            nc.sync.dma_start(
                                od[
                                    self.b,
                                    fc * 128 : (fc + 1) * 128,
                                    self.i0 + OUT_R1 : self.i0 + S,
                                    :,
                                ],
                                self.ot[:, fc, OUT_R1 * OW :],
                            )

                def flush(self):
                    self.emit(S * OW + 1)

            prev_d = None  # previous strip's StripD with tail tiles pending

            ordinal = 0
            for b in range(BLOC):
                for i0, _S in STRIPS:
                    first = ordinal == 0
                    ordinal += 1

                    xs_t = xs_pool.tile([128, 2, NRI * W], f16)
                    if first:
                        # split the first load so tile-0 matmuls start early
                        for ch in range(2):
                            nc.gpsimd.dma_start(
                                xs_t[:, ch, 0 : 12 * W],
                                xd[b, ch * 128 : (ch + 1) * 128, 0:12, :],
                            )
                        for ch in range(2):
                            nc.gpsimd.dma_start(
                                xs_t[:, ch, 12 * W :],
                                xd[b, ch * 128 : (ch + 1) * 128, 12:NRI, :],
                            )
                    else:
                        for ch in range(2):
                            nc.gpsimd.dma_start(
                                xs_t[:, ch, :],
                                xd[b, ch * 128 : (ch + 1) * 128, i0 : i0 + NRI, :],
                            )

                    y3_t = y3_pool.tile([128, S * OW], f16)
                    ot = osb_pool.tile([128, 4, S * OW], f16)
                    cur_d = StripD(b, i0, y3_t, ot)

                    r0 = 0
                    for t_idx, nr in enumerate(FOLD_ROWTILES):
                        ncols = nr * W
                        pa = psa_pool.tile([128, 512], f32, tag="pa")
                        k = 0
                        for ch in range(2):
                            for h in range(FH):
                                nc.tensor.matmul(
                                    pa[:, 0:ncols],
                                    wt_sb[:, h * 2 + ch, :],
                                    xs_t[
                                        :,
                                        ch,
                                        (r0 + h) * W : (r0 + h) * W + ncols,
                                    ],
                                    start=(k == 0),
                                    stop=(k == 5),
                                )
                                k += 1
                        pav = pa[:, 0:ncols].rearrange("p (r w) -> p r w", w=W)
                        dst = y3_t[:, r0 * OW : (r0 + nr) * OW].rearrange(
                            "p (r j) -> p r j", j=OW
                        )
                        nc.scalar.mul(dst, pav[:, :, 0:OW], wsc_sb[:, 3:4])
                        nc.vector.scalar_tensor_tensor(
                            dst, pav[:, :, 1 : 1 + OW], wsc_sb[:, 4:5],
                            dst, op0=mult, op1=add,
                        )
                        nc.vector.scalar_tensor_tensor(
                            dst, pav[:, :, 2 : 2 + OW], wsc_sb[:, 5:6],
                            dst, op0=mult, op1=add,
                        )
                        # flush the previous strip's D tail once this strip's
                        # pipeline is primed
                        if t_idx == 1 and prev_d is not None:
                            prev_d.flush()
                            prev_d = None
                        # emit D tiles one row-tile behind their producers
                        cur_d.emit(r0 * OW)
                        r0 += nr

                    prev_d = cur_d

            if prev_d is not None:
                prev_d.flush()

    nc.compile()
    return nc


def _get_nc():
    if "nc" not in _NC_CACHE:
        _NC_CACHE["nc"] = _build_nc()
    return _NC_CACHE["nc"]


def _prep_weights(factor0, factor1, factor2, factor3):
    wa = (factor3[None, :, :] * factor1[:, None, :]).reshape(FH, 2, 128, R)
    w3 = factor3.reshape(2, 128, R)
    w0 = factor0.reshape(4, 128, R).transpose(0, 2, 1)
    wt = np.concatenate(
        [wa.reshape(6, 128, R), w3, w0], axis=0
    ).astype(np.float16)
    wt = np.ascontiguousarray(wt.transpose(1, 0, 2))
    wsc = np.zeros((R, 8), dtype=np.float32)
    wsc[:, 0:3] = factor1.T
    wsc[:, 3:6] = factor2.T
    return wt, wsc


def _prep_x(x):
    return np.ascontiguousarray(x).astype(np.float16)


def _make_in_maps(x, factor0, factor1, factor2, factor3):
    wt, wsc = _prep_weights(factor0, factor1, factor2, factor3)
    x16 = _prep_x(x)
    return [
        {"x": x16[c * BLOC : (c + 1) * BLOC], "wt": wt, "wsc": wsc}
        for c in range(NCORES)
    ]


def kernel(x, factor0, factor1, factor2, factor3):
    from concourse import bass_utils

    x = np.asarray(x, dtype=np.float32)
    factor0 = np.asarray(factor0, dtype=np.float32)
    factor1 = np.asarray(factor1, dtype=np.float32)
    factor2 = np.asarray(factor2, dtype=np.float32)
    factor3 = np.asarray(factor3, dtype=np.float32)

    in_maps = _make_in_maps(x, factor0, factor1, factor2, factor3)
    nc = _get_nc()
    res = bass_utils.run_bass_kernel_spmd(nc, in_maps, list(range(NCORES)))
    out = np.concatenate(
        [res.results[c]["out"] for c in range(NCORES)], axis=0
    )
    return out.astype(np.float32)
